# revision 25
# baseline (speedup 1.0000x reference)
"""Trainium2 Bass kernel for masked-softmax attention scoring.

Reference computation (B=128, T=512, K=1024, Q=1024):
    mids  = einsum("kq,bq->bk", W, query)
    s     = tanh(einsum("btk,bk->bt", key, mids) + bias)
    attn  = softmax-like: exp(s - max) * mask / sum(exp(s - max) * mask)

The max-subtraction cancels exactly in the ratio (tanh is bounded), so the
device computes  attn = exp(tanh(.)) * mask / sum_t(exp(tanh(.)) * mask).

Sharding: data-parallel over B across 8 NeuronCores (16 batches/core).

v16 design ("masked row packing" + piece-wise full-kc streaming):
  * HBM-streaming bound: the stream sustains ~359 GB/s with zero mid
    gaps, so the levers are fewer bytes and a shorter post-stream tail.
  * Row packing: scores are needed only where mask==1 (~80% of rows).
    The host packs kept key rows using a template uniform across the 8
    SPMD cores (per-batch region = max kept over cores).  One group of
    all 16 batches packs 6784 rows = exactly 53 blocks of 128 (vs 54
    with two groups).  Key traffic 16.78 -> 13.9 MB/core.
  * Piece-wise FULL-KC streaming (the v16 change): each DMA piece
    carries ALL 8 k-chunks for a 6-block column range (kc-minor layout,
    ~12.3KB/partition runs).  A block's scores complete as soon as its
    piece lands, so tanh/exp/sel/colsums for pieces 0..N-2 run DURING
    the stream; only the last (5-block) piece's work plus the final
    denominator join remains after the last HBM byte.  (With kc-major
    pieces every block finished at stream end and the whole chain was
    serial tail.)
  * Per block j, per kc: one 128x128 fp16 stationary load (key rows)
    and a 2-column moving pass with mids[:, bj:bj+2]; the 8 kc matmuls
    per column are consecutive (per-column PSUM accumulation groups).
  * Epilogue: tanh (+bias, from PSUM) -> exp fp16 -> sel-mask multiply
    (zeroes wrong-batch columns / pad rows), all per-piece slices; per
    piece one colsums matmul (em slice as stationary, ones column) into
    disjoint partitions of one PSUM bank; then once: cast -> denom
    matmul (0/1 column->batch map stationary) -> reciprocal -> mapT *
    rden -> rank-1 broadcast matmul -> final scale -> one fp16 out DMA.
    Wrong-column / pad entries are dropped by the host-side scatter.
  * DMA rings: sync/scalar alternate key pieces (enqueue order ==
    consumption order per ring); GpSimd software ring carries small
    loads; the output DMA goes at the end of the sync program.
  * Chain PE matmuls (colsums) are emitted with a one-piece lag so PE
    never head-of-line blocks on ScalarE/DVE chain ops.
  * fp16 wire format (key, W^T): raw scores std ~59, tanh saturates;
    measured rel-l2 ~1.4e-3 vs the 2e-2 budget (fp8 fails: ~4.5e-2).
"""

import sys

if "/opt/trn_rl_repo" not in sys.path:
    sys.path.insert(0, "/opt/trn_rl_repo")

from contextlib import ExitStack

import numpy as np

# ---- problem constants (hardcoded per spec) ----
B, T, K, Q = 128, 512, 1024, 1024
NCORES = 8
BS = B // NCORES          # 16 batches per core
P = 128                   # SBUF partitions
KC = K // P               # 8 contraction chunks for the scores matmuls
QC = Q // P               # 8 contraction chunks for the mids matmuls
MB = BS + 1               # mids batch columns (1 zero pad for block pairs)

PIECE_NB = 6              # blocks per full-kc streaming piece
KP_BUFS = 5               # piece pool depth

_STATE: dict = {}


def _plan_from_mask(mask):
    """Template packing plan shared by all 8 cores (SPMD: one program)."""
    kept = mask.sum(axis=1).astype(np.int64).reshape(NCORES, BS)
    tmpl_len = kept.max(axis=0)          # rows reserved per local batch
    assert tmpl_len.min() >= P, "packing assumes every batch keeps >=128 rows"
    starts = np.concatenate([[0], np.cumsum(tmpl_len)])
    NB = int(np.ceil(starts[-1] / P))
    assert 2 * NB <= P, "em stationary must fit 128 columns"
    bj = tuple(
        min(int(np.searchsorted(starts, j * P, side="right")) - 1, BS - 2)
        for j in range(NB)
    )
    return starts, NB, bj


def _build_nc(plan):
    import concourse.tile as tile
    from concourse import bacc, mybir

    f32 = mybir.dt.float32
    f16 = mybir.dt.float16
    nc = bacc.Bacc()

    starts, NB, bj = plan
    NB2 = NB * 2
    NFULL = (NB - 1) // PIECE_NB            # full 6-block pieces
    LASTNB = NB - NFULL * PIECE_NB          # blocks in last piece (1..6)
    NP = NFULL + 1

    kpp_e = nc.declare_dram_parameter(
        "kpp", [NFULL, P, KC, PIECE_NB * P], f16, isOutput=False
    )
    kpl_e = nc.declare_dram_parameter(
        "kpl", [P, KC, LASTNB * P], f16, isOutput=False
    )
    # wt[h, qp, qh, kc, kl] = W[kc*128 + kl, (h*4 + qh)*128 + qp]
    wt_e = nc.declare_dram_parameter("wt", [2, P, QC // 2, KC, P], f16, isOutput=False)
    qt_e = nc.declare_dram_parameter("qt", [P, QC, MB], f16, isOutput=False)
    bias_e = nc.declare_dram_parameter("biasb", [P, 1], f32, isOutput=False)
    sel_e = nc.declare_dram_parameter("sel0", [P, NB2], f16, isOutput=False)
    map_e = nc.declare_dram_parameter(
        "map0", [PIECE_NB * 2, NP, BS], f16, isOutput=False
    )
    mapT_e = nc.declare_dram_parameter("mapT0", [BS, NB2], f16, isOutput=False)
    out_e = nc.declare_dram_parameter("out0", [P, NB2], f16, isOutput=True)

    with tile.TileContext(nc) as tc, ExitStack() as ctx:
        const = ctx.enter_context(tc.tile_pool(name="const", bufs=1))
        kpool = ctx.enter_context(tc.tile_pool(name="key", bufs=KP_BUFS))
        kpooll = ctx.enter_context(tc.tile_pool(name="keyl", bufs=1))
        psum = ctx.enter_context(tc.tile_pool(name="psum", bufs=1, space="PSUM"))

        rings = [nc.sync, nc.scalar]

        wt_sbs = [
            const.tile([P, QC // 2, KC, P], f16, tag=f"wt{h}", name=f"wt{h}")
            for h in range(2)
        ]
        nc.sync.dma_start(out=wt_sbs[0][:], in_=wt_e[0])
        nc.scalar.dma_start(out=wt_sbs[1][:], in_=wt_e[1])

        qt_sb = const.tile([P, QC, MB], f16)
        bias_sb = const.tile([P, 1], f32)
        sel_sb = const.tile([P, NB2], f16)
        map_sb = const.tile([PIECE_NB * 2, NP, BS], f16)
        mapT_sb = const.tile([BS, NB2], f16)
        nc.gpsimd.dma_start(out=qt_sb[:], in_=qt_e[:])
        nc.gpsimd.dma_start(out=bias_sb[:], in_=bias_e[:])
        nc.gpsimd.dma_start(out=sel_sb[:], in_=sel_e[:])
        nc.gpsimd.dma_start(out=map_sb[:], in_=map_e[:])
        nc.gpsimd.dma_start(out=mapT_sb[:], in_=mapT_e[:])

        ones_col = const.tile([P, 1], f16)
        nc.vector.memset(ones_col[:], 1.0)
        ones_bat = const.tile([BS, P], f16)
        nc.vector.memset(ones_bat[:], 1.0)

        # all piece dma_starts upfront; pool rotation paces the sequencers
        pieces = []
        for p in range(NFULL):
            t = kpool.tile([P, KC, PIECE_NB * P], f16, tag="kp", name=f"kp{p}")
            rings[p % 2].dma_start(out=t[:], in_=kpp_e[p])
            pieces.append(t)
        tl = kpooll.tile([P, KC, LASTNB * P], f16, tag="kl", name="kl")
        rings[NFULL % 2].dma_start(out=tl[:], in_=kpl_e[:])
        pieces.append(tl)

        # ---- mids^T[k, (kc, b)] = sum_q W[k, q] query[b, q] (pad col) ----
        mids_ps = psum.tile([P, KC, MB], f32)
        for qi, (h, qh) in enumerate(
            [(0, 0), (0, 1), (0, 2), (0, 3), (1, 0), (1, 1), (1, 2), (1, 3)]
        ):
            for kc in range(KC):
                nc.tensor.matmul(
                    mids_ps[:, kc, :],
                    lhsT=wt_sbs[h][:, qh, kc, :],
                    rhs=qt_sb[:, h * (QC // 2) + qh, :],
                    start=(qi == 0 and kc == 0),
                    stop=(qi == QC - 1),
                )
        mids_sb = const.tile([P, KC, MB], f16)
        nc.vector.tensor_copy(mids_sb[:], mids_ps[:])

        # ---- piece-wise streaming + per-piece epilogue ----
        sc_ps = psum.tile([P, NB2], f32, tag="sc", name="sc")
        th = const.tile([P, NB2], f32, tag="th", name="th")
        ex = const.tile([P, NB2], f16, tag="ex", name="ex")
        em = const.tile([P, NB2], f16, tag="em", name="em")
        cs_ps = psum.tile([P, NP], f32, tag="cs", name="cs")
        cs_sb = const.tile([P, NP], f16, tag="csb", name="csb")
        dn_ps = psum.tile([BS, 1], f32, tag="dn", name="dn")

        def piece_mms(p):
            j0 = p * PIECE_NB
            j1 = min(j0 + PIECE_NB, NB)
            kt = pieces[p]
            for j in range(j0, j1):
                mb0 = bj[j]
                for kc in range(KC):
                    nc.tensor.matmul(
                        sc_ps[:, 2 * j : 2 * j + 2],
                        lhsT=kt[:, kc, (j - j0) * P : (j - j0 + 1) * P],
                        rhs=mids_sb[:, kc, mb0 : mb0 + 2],
                        start=(kc == 0),
                        stop=(kc == KC - 1),
                    )

        def piece_pre(p):
            # tanh/exp/sel for piece p's column slice (ScalarE + DVE)
            j0 = p * PIECE_NB
            j1 = min(j0 + PIECE_NB, NB)
            sl = slice(2 * j0, 2 * j1)
            nc.scalar.activation(
                out=th[:, sl],
                in_=sc_ps[:, sl],
                func=mybir.ActivationFunctionType.Tanh,
                bias=bias_sb[:],
                scale=1.0,
            )
            nc.scalar.activation(
                out=ex[:, sl], in_=th[:, sl], func=mybir.ActivationFunctionType.Exp
            )
            nc.vector.tensor_tensor(
                em[:, sl], ex[:, sl], sel_sb[:, sl], mybir.AluOpType.mult
            )

        def piece_cs(p):
            # colsums for piece p -> column p of the cs bank (partition 0
            # base: matmul PSUM dst must start at partition 0/32/64), then
            # fp16 copy and an ACCUMULATING per-piece denominator matmul —
            # so all but the last piece's denominator work streams too.
            j0 = p * PIECE_NB
            j1 = min(j0 + PIECE_NB, NB)
            w = 2 * (j1 - j0)
            nc.tensor.matmul(
                cs_ps[:w, p : p + 1],
                lhsT=em[:, 2 * j0 : 2 * j1],
                rhs=ones_col[:],
                start=True,
                stop=True,
            )
            nc.vector.tensor_copy(cs_sb[:w, p : p + 1], cs_ps[:w, p : p + 1])
            nc.tensor.matmul(
                dn_ps[:],
                lhsT=map_sb[:w, p, :],
                rhs=cs_sb[:w, p : p + 1],
                start=(p == 0),
                stop=(p == NP - 1),
            )

        for p in range(NP):
            piece_mms(p)
            piece_pre(p)
            if p >= 1:
                piece_cs(p - 1)     # one-piece lag: no PE head-of-line stall
        piece_cs(NP - 1)

        # ---- final join: reciprocal, broadcast, scale ----
        rden = const.tile([BS, 1], f16, tag="rd", name="rd")
        with nc.allow_low_precision(reason="1/denom fp16: rel 5e-4 << 2e-2"):
            nc.vector.reciprocal(out=rden[:], in_=dn_ps[:])
        rdmap = const.tile([BS, NB2], f16, tag="rm", name="rm")
        nc.vector.tensor_tensor(
            rdmap[:], mapT_sb[:], rden[:].broadcast_to((BS, NB2)),
            mybir.AluOpType.mult,
        )
        rb_ps = psum.tile([P, NB2], f32, tag="rb", name="rb")
        nc.tensor.matmul(
            rb_ps[:], lhsT=ones_bat[:], rhs=rdmap[:], start=True, stop=True
        )
        attn = const.tile([P, NB2], f16, tag="at", name="at")
        with nc.allow_low_precision(reason="attn fp16 out: rel 5e-4 << 2e-2"):
            nc.vector.tensor_tensor(attn[:], em[:], rb_ps[:], mybir.AluOpType.mult)
        nc.sync.dma_start(out=out_e[:], in_=attn[:])

    nc.compile()
    return nc


def _get_nc(plan):
    key = (plan[1], plan[2])
    if _STATE.get("key") != key:
        _STATE["nc"] = _build_nc(plan)
        _STATE["key"] = key
    return _STATE["nc"]


def _make_in_maps(query, key, mask, W, bias):
    query = np.asarray(query, dtype=np.float32)
    key = np.asarray(key, dtype=np.float32)
    mask = np.asarray(mask, dtype=np.float32)
    W = np.asarray(W, dtype=np.float32)
    bias = np.asarray(bias, dtype=np.float32).reshape(-1)

    plan = _plan_from_mask(mask)
    _STATE["plan"] = plan
    starts, NB, bj = plan
    NB2 = NB * 2
    NFULL = (NB - 1) // PIECE_NB
    LASTNB = NB - NFULL * PIECE_NB
    R = NB * P

    WT = np.ascontiguousarray(
        W.T.astype(np.float16).reshape(2, QC // 2, P, KC, P).transpose(0, 2, 1, 3, 4)
    )
    biasb = np.ascontiguousarray(
        np.broadcast_to(bias[:1][None, :], (P, 1)).astype(np.float32)
    )
    key16 = key.astype(np.float16)

    mp = np.zeros((NB2, BS), np.float16)
    for j in range(NB):
        for cc in range(2):
            bb = bj[j] + cc
            if bb < BS:
                mp[2 * j + cc, bb] = 1.0
    mpT = np.ascontiguousarray(mp.T)
    NP = NFULL + 1
    mpP = np.zeros((PIECE_NB * 2, NP, BS), np.float16)
    for p in range(NP):
        j0, j1 = p * PIECE_NB, min((p + 1) * PIECE_NB, NB)
        mpP[: 2 * (j1 - j0), p, :] = mp[2 * j0 : 2 * j1, :]

    in_maps = []
    scatter = []
    for c in range(NCORES):
        m = {"wt": WT, "biasb": biasb, "map0": mpP, "mapT0": mpT}
        qh = query[c * BS : (c + 1) * BS].T.astype(np.float16)
        qtp = np.zeros((Q, MB), np.float16)
        qtp[:, :BS] = qh
        m["qt"] = np.ascontiguousarray(qtp.reshape(QC, P, MB).transpose(1, 0, 2))
        buf = np.zeros((R, K), np.float16)
        sel = np.zeros((P, NB, 2), np.float16)
        r_list, lb_list, t_list = [], [], []
        for lb in range(BS):
            gb = c * BS + lb
            ts = np.nonzero(mask[gb])[0]
            r0 = int(starts[lb])
            rr = r0 + np.arange(len(ts))
            buf[rr] = key16[gb, ts]
            jj, tp = rr // P, rr % P
            cc = lb - np.asarray(bj)[jj]
            sel[tp, jj, cc] = 1.0
            r_list.append(rr); lb_list.append(np.full(len(ts), lb)); t_list.append(ts)
        keyt = np.ascontiguousarray(buf.T).reshape(KC, P, R)
        # full pieces: [NFULL, P, KC, PIECE_NB*P] (kc-minor per piece)
        CW = PIECE_NB * P
        m["kpp"] = np.ascontiguousarray(
            keyt[:, :, : NFULL * CW]
            .reshape(KC, P, NFULL, CW)
            .transpose(2, 1, 0, 3)
        )
        m["kpl"] = np.ascontiguousarray(
            keyt[:, :, NFULL * CW :].transpose(1, 0, 2)
        )
        m["sel0"] = np.ascontiguousarray(sel.reshape(P, NB2))
        in_maps.append(m)
        scatter.append(
            (np.concatenate(r_list), np.concatenate(lb_list), np.concatenate(t_list))
        )
    _STATE["scatter"] = scatter
    return in_maps


def _run(in_maps, **kwargs):
    from concourse.bass_utils import run_bass_kernel_spmd

    return run_bass_kernel_spmd(
        _get_nc(_STATE["plan"]), in_maps, core_ids=list(range(NCORES)), **kwargs
    )


def _gather(results):
    starts, NB, bj = _STATE["plan"]
    scatter = _STATE["scatter"]
    attn = np.zeros((B, T), dtype=np.float32)
    for c, r in enumerate(results):
        out = np.asarray(r["out0"])
        rr, lb, ts = scatter[c]
        jj, tp = rr // P, rr % P
        cc = lb - np.asarray(bj)[jj]
        attn[c * BS + lb, ts] = out[tp, 2 * jj + cc]
    return attn


def kernel(query, key, mask, W, bias):
    in_maps = _make_in_maps(query, key, mask, W, bias)
    res = _run(in_maps)
    return _gather(res.results)
